# revision 35
# baseline (speedup 1.0000x reference)
"""ChebConv (K=4) distributed Trainium2 kernel — 8 NeuronCores.

Strategy:
  - x (B,Cin,V) is reshaped host-side to a node-major feature table
    x0[(V, F=B*Cin)] (bf16) and replicated to all 8 cores as the gather
    table for spmm step 1.
  - Edges are sorted by destination row; rows are sharded across the 8
    cores (6272 rows each, V padded to 50176). Each core processes only
    the edges landing in its rows.
  - Each spmm step: dma_gather (MoE gather instruction, 1KB descriptors)
    pulls x[col] rows for chunks of 128 edges; a per-chunk segment
    matrix M[e, r] = val[e] * (row[e] == r) is built on the vector
    engine; PE matmul M^T @ z accumulates each 128-row tile in PSUM.
    The Chebyshev recurrence (x2 = 2*A@x1 - x0) folds the factor 2 into
    the edge values and becomes a single subtract in the epilogue.
  - After steps 1 and 2 an 8-core AllGather rebuilds the full node table
    (the "halo exchange") used as the next step's gather source.
  - The K-contraction einsum runs per row-tile right after step 3:
    PE-transpose of each (v,c) tile then matmul against the replicated
    weights, bias added on the scalar engine, f32 result DMA'd out.

int16 gather indices limit a table to 32768 rows, so the node table is
split in two halves at a tile-aligned quarter boundary; edges are
grouped host-side by (row-tile, col-half) and padded to multiples of
128. Per-step AllGathers are chunked over tile-aligned quarter row
ranges and emitted inline at quarter boundaries so they overlap the
remaining gathers; the k=0,1 half of the output einsum runs in step 1's
shadow, k=2,3 and the combine in step 3's epilogue.
"""

import os
import numpy as np
import ml_dtypes

import concourse.bacc as bacc
import concourse.bass as bass
import concourse.mybir as mybir
import concourse.tile as tile
import concourse.tile_sem_assignment as tsa
from concourse.bass_utils import run_bass_kernel_spmd

# Tile assigns DMASW semaphore lanes round-robin regardless of the SWDGE
# queue an instruction runs on. With multi-queue gathers that mixes
# completions from different queues on one lane semaphore (out-of-order
# increments vs. the lane's FIFO wait thresholds). Pin queue-tagged Pool
# DMAs to lanes {2q, 2q+1} so every lane semaphore is fed by exactly one
# queue (per-queue completions are FIFO).
if not getattr(tsa.TileClockTick, "_cheb_queue_lanes", False):
    _orig_assign_tick = tsa.TileClockTick._assign_tick

    def _assign_tick_queue_aware(self, inst):
        q = getattr(inst, "queue_num", None)
        if (
            # multi-queue mode pins every queue (incl. 0); single-queue
            # keeps Tile's native lane rotation for queue 0
            (q or (q == 0 and NQUEUES > 1))
            and inst.engine == mybir.EngineType.Pool
            and isinstance(inst, tsa.DMAInst)
        ):
            toggles = getattr(self, "_cheb_q_toggle", None)
            if toggles is None:
                toggles = self._cheb_q_toggle = {}
            t = toggles.get(q, 0)
            toggles[q] = t ^ 1
            lane = (2 * q + t) % self.swdge_sem_count
            saved = self.next_sw_dma_idx
            self.next_sw_dma_idx = lane
            try:
                return _orig_assign_tick(self, inst)
            finally:
                self.next_sw_dma_idx = saved
        return _orig_assign_tick(self, inst)

    tsa.TileClockTick._assign_tick = _assign_tick_queue_aware
    tsa.TileClockTick._cheb_queue_lanes = True

# ----- problem constants (hardcoded per spec) -----
V = 50000
B = 4
CIN = 128
COUT = 128
K = 4
E = 800000
F = B * CIN  # 512
NCORES = 8
TILES_PER_CORE = 49
R = TILES_PER_CORE * 128          # 6272 rows per core
V_PAD = NCORES * R                # 50176
# lo/hi gather-table split (int16 idx limit) computed from quarter bounds

MAX_BLK = int(os.environ.get("CHEB_MAXBLK", "8"))  # chunks per gather block
NQUEUES = int(os.environ.get("CHEB_NQ", "1"))      # SWDGE queues for gathers
SCRATCH = int(os.environ.get("CHEB_SCRATCH", "16384"))  # SWDGE ring bytes/queue
# -1-pad descriptor trimming is OFF by default: the gather ucode trims
# trailing -1 idxs, but the instruction decode reserves descriptor-ring
# space from the num_idxs REGISTER — with the compile-time slot count in
# the register the reservation exceeds the pushed count and the ring
# accounting leaks until the queue deadlocks.  Trimming is only sound
# with per-core reg_load of exact valid counts (costs more than it saves).
TRIM = os.environ.get("CHEB_TRIM", "") == "1"


def _quarter_rows():
    """Tile-aligned AllGather chunk boundaries (local rows).

    The last chunk is deliberately small: it is emitted after the step's
    final epilogue and directly gates the next step's hi-half gathers.
    """
    if TILES_PER_CORE == 49:
        # lo half = first two quarters = 32 tiles = exactly 32768 table
        # rows (max int16-addressable); tiny last quarter so the final,
        # boundary-gating AllGather is short
        qb = [0, 16 * 128, 32 * 128, 45 * 128, 49 * 128]
        return np.array(qb, dtype=np.int64)
    nag = min(4, TILES_PER_CORE)
    base, rem = divmod(TILES_PER_CORE, nag)
    qb = [0]
    for q in range(nag):
        qb.append(qb[-1] + (base + (1 if q < rem else 0)) * 128)
    return np.array(qb, dtype=np.int64)


def _half_rows():
    """Gather-table row where the lo/hi (int16) table split falls."""
    rb = _quarter_rows()
    return NCORES * int(rb[(len(rb) - 1 + 1) // 2])


def _table_row(v):
    """Node id -> gather-table row.

    The per-step AllGather runs as chunked collectives over tile-aligned
    quarter row ranges. Each chunk's output is rank-major and must be
    contiguous, so the table row order is (quarter, core, offset).
    """
    rb = _quarter_rows()
    r = v // R
    l = v % R
    q = np.searchsorted(rb, l, side="right") - 1
    qlen = rb[q + 1] - rb[q]
    return NCORES * rb[q] + r * qlen + (l - rb[q])
DEBUG_NOCC = os.environ.get("CHEB_NOCC", "") == "1"    # skip collectives
DEBUG_STEPS = int(os.environ.get("CHEB_STEPS", "3"))   # spmm steps to run

USE_BF16 = os.environ.get("CHEB_F32", "") != "1"
DT = mybir.dt.bfloat16 if USE_BF16 else mybir.dt.float32
NPDT = ml_dtypes.bfloat16 if USE_BF16 else np.float32

LAST_RESULT = None  # test harness reads exec_time_ns from here


# ---------------------------------------------------------------------------
# host-side edge preprocessing
# ---------------------------------------------------------------------------

class _Block:
    __slots__ = ("half", "icol", "cg0", "n", "chunks", "nidx", "queue")

    def __init__(self, half, icol, cg0, n, chunks, nidx):
        self.queue = 0
        self.half = half      # 0 = cols [0, HALF), 1 = cols [HALF, V_PAD)
        self.icol = icol      # column offset into the idx sbuf tensor
        self.cg0 = cg0        # first global chunk id of this block
        self.n = n            # number of 128-edge chunks
        self.chunks = chunks  # list of (tile, is_first_of_tile, is_last_of_tile)
        self.nidx = nidx      # index slots (<= n*128); actual gathered count
                              # comes from the per-core counts register


def _preprocess(edge_row, edge_col, edge_vals):
    """Group/pad edges per (core, row-tile, col-half).

    Chunk counts are equalized across cores (max) so all cores run the
    same instruction graph.  One gather block per (tile, half) bucket:
    short cores pad with trailing idx=-1 slots, which the gather ucode
    trims (no descriptors generated); the per-core valid count is
    shipped in `cnts` and loaded into the gather's num_idxs register at
    runtime.  Returns (blocks, NCH, IDXCOLS, cnts, per-core arrays).
    """
    core = edge_row // R
    tile_id = (edge_row % R) // 128
    tcol = _table_row(edge_col)
    half_split = _half_rows()
    half = (tcol >= half_split).astype(np.int64)

    # bucket edges by (core, tile, half)
    key = (core * TILES_PER_CORE + tile_id) * 2 + half
    order = np.argsort(key, kind="stable")
    skey = key[order]
    srow = edge_row[order]
    scol = tcol[order]
    sval = edge_vals[order]
    nkeys = NCORES * TILES_PER_CORE * 2
    counts = np.bincount(skey, minlength=nkeys).reshape(NCORES, TILES_PER_CORE, 2)
    starts = np.zeros(nkeys + 1, dtype=np.int64)
    np.cumsum(counts.reshape(-1), out=starts[1:])

    # chunks per (tile, half): max over cores, >=1 each
    nchunk = np.ceil(counts / 128.0).astype(np.int64).max(axis=0)  # (T, 2)
    nchunk = np.maximum(nchunk, 1)

    # Phase-major chunk order: ALL hi-col blocks first (their gather
    # table is the one the previous step's late AllGather chunks feed —
    # by the time this step starts they have landed), then all lo-col
    # blocks.  Within each phase, groups owning the hi-table rows
    # (tiles >= 32) come first so their AllGather chunks fire mid-step
    # and land before the NEXT step's hi phase.  Tiles keep 4-tile
    # groups so the psum pool rotation stays within its 4 buffers.
    # Buckets larger than BLK_CAP chunks split into parts (descriptor
    # ring capacity caps one gather at ~1024 descriptors).
    tile_seq = list(range(32, TILES_PER_CORE)) + list(range(0, 32))
    order = [(t, h) for h in (1, 0) for t in tile_seq]
    chunk_list = []    # (tile, half) per global chunk, in order
    part_list = {}     # (t, h) -> list of (cg0, part_chunks)
    for (t, h) in order:
        n = int(nchunk[t, h])
        cg0 = len(chunk_list)
        part_list[(t, h)] = [(cg0, n)]
        chunk_list.extend([(t, h)] * n)

    # cut each phase's chunk run into MAX_BLK-sized gather blocks (a
    # block's chunks share one source table = one half, guaranteed since
    # phases are half-major)
    blocks = []
    i = 0
    while i < len(chunk_list):
        h = chunk_list[i][1]
        j = i
        while j < len(chunk_list) and j - i < MAX_BLK and chunk_list[j][1] == h:
            j += 1
        blocks.append(_Block(h, i * 8, i, j - i,
                             [chunk_list[k][0] for k in range(i, j)],
                             (j - i) * 128))
        i = j

    NCH = len(chunk_list)
    IDXCOLS = NCH * 8

    # greedy queue assignment: keep the 4 Q7 pairs' descriptor loads
    # balanced in every 4-block window (round-robin would pair small
    # hi buckets onto the same queue)
    qload = [0] * max(NQUEUES, 1)
    for blk in blocks:
        q = min(range(len(qload)), key=lambda i: qload[i])
        blk.queue = q
        qload[q] += blk.n

    # mark first/last chunk per (tile, half) bucket: `first` opens the
    # tile's psum for the phase, `last` closes it (hi: partial spill,
    # lo: full epilogue)
    bucket_first = {k: parts[0][0] for k, parts in part_list.items()}
    bucket_last = {k: parts[0][0] + parts[0][1] - 1
                   for k, parts in part_list.items()}
    for blk in blocks:
        marked = []
        h = blk.half
        for j, t in enumerate(blk.chunks):
            i = blk.cg0 + j
            marked.append((t, i == bucket_first[(t, h)],
                           i == bucket_last[(t, h)]))
        blk.chunks = marked

    # per-core packed arrays
    per_core = []
    cnts = np.zeros((NCORES, len(blocks)), dtype=np.int32)
    for c in range(NCORES):
        idx_np = np.zeros((128, IDXCOLS), dtype=np.int16)
        val_np = np.zeros((128, NCH), dtype=np.float32)
        roff_np = np.zeros((128, NCH), dtype=np.float32)
        for t in range(TILES_PER_CORE):
            base_row = c * R + t * 128
            for h in range(2):
                kidx = (c * TILES_PER_CORE + t) * 2 + h
                s, e = starts[kidx], starts[kidx + 1]
                col = scol[s:e].astype(np.int64) - h * half_split
                row = srow[s:e].astype(np.int64) - base_row
                val = sval[s:e].astype(np.float32)
                pos = 0
                for cg0p, pn in part_list[(t, h)]:
                    cap = pn * 128
                    ccol = col[pos:pos + cap]
                    crow = row[pos:pos + cap]
                    cval = val[pos:pos + cap]
                    pos += len(ccol)
                    cnt = len(ccol)
                    if cnt == 0:
                        # gather one dummy row so num_idxs stays > 0
                        ccol = np.zeros(1, dtype=np.int64)
                        crow = np.zeros(1, dtype=np.int64)
                        cval = np.zeros(1, dtype=np.float32)
                        cnt = 1
                    pad = cap - cnt
                    if pad:
                        ccol = np.concatenate(
                            [ccol, np.zeros(pad, dtype=np.int64)])
                        crow = np.concatenate(
                            [crow, np.zeros(pad, dtype=np.int64)])
                        cval = np.concatenate(
                            [cval, np.zeros(pad, dtype=np.float32)])
                    per_core_write(idx_np, val_np, roff_np, cg0p,
                                   ccol, crow, cval)
        per_core.append((idx_np, val_np, roff_np))
    return blocks, NCH, IDXCOLS, cnts, per_core


def per_core_write(idx_np, val_np, roff_np, cg0, col, row, val):
    # idx layout per dma_gather: within a (t,h) group starting at global
    # chunk cg0 (icol = cg0*8), edge i -> col cg0*8 + i//16, partition
    # (i%16) + 16*g replicated across the 8 gpsimd core groups g.
    n128 = col.shape[0]
    i = np.arange(n128)
    cols = cg0 * 8 + i // 16
    parts = i % 16
    for g in range(8):
        idx_np[parts + 16 * g, cols] = col.astype(np.int16)
    # val/rowoff layout: (partition=edge%128, col=global chunk id)
    ch = cg0 + i // 128
    p = i % 128
    val_np[p, ch] = val
    roff_np[p, ch] = row.astype(np.float32)


# ---------------------------------------------------------------------------
# device graph
# ---------------------------------------------------------------------------

def _build_nc(blocks, NCH, IDXCOLS):
    nc = bacc.Bacc("TRN2", target_bir_lowering=False, debug=False,
                   num_devices=NCORES, num_swdge_queues=NQUEUES,
                   dynamic_dma_scratch_size=SCRATCH)
    f32 = mybir.dt.float32
    NBLK = len(blocks)

    # ---- I/O ----
    x0_tab = nc.dram_tensor("x0_tab", [V_PAD, F], DT, kind="ExternalInput")
    x0_own = nc.dram_tensor("x0_own", [R, F], DT, kind="ExternalInput")
    idxs_d = nc.dram_tensor("idxs", [128, IDXCOLS], mybir.dt.int16,
                            kind="ExternalInput")
    val1_d = nc.dram_tensor("val1", [128, NCH], DT, kind="ExternalInput")
    val2_d = nc.dram_tensor("val2", [128, NCH], DT, kind="ExternalInput")
    roff_d = nc.dram_tensor("roff", [128, NCH], DT, kind="ExternalInput")
    iota_d = nc.dram_tensor("iota", [128, 128], DT, kind="ExternalInput")
    ident_d = nc.dram_tensor("ident", [128, 128], DT, kind="ExternalInput")
    w_d = nc.dram_tensor("w", [CIN, K * COUT], DT, kind="ExternalInput")
    bias_d = nc.dram_tensor("bias", [COUT, 1], f32, kind="ExternalInput")
    out_d = nc.dram_tensor("out", [B, COUT, R], f32, kind="ExternalOutput")

    # ---- internal DRAM ----
    x1_own_d = nc.dram_tensor("x1_own_d", [R, F], DT)
    x2_own_d = nc.dram_tensor("x2_own_d", [R, F], DT)
    # lo/hi halves as separate tensors so lo-half gathers only depend on
    # the AllGather chunks that write them (Tile tracks DRAM deps per
    # tensor, not per range)
    HS = _half_rows()
    x1_lo = nc.dram_tensor("x1_lo", [HS, F], DT, addr_space="Shared")
    x1_hi = nc.dram_tensor("x1_hi", [V_PAD - HS, F], DT, addr_space="Shared")
    x2_lo = nc.dram_tensor("x2_lo", [HS, F], DT, addr_space="Shared")
    x2_hi = nc.dram_tensor("x2_hi", [V_PAD - HS, F], DT, addr_space="Shared")

    rg = [list(range(NCORES))]

    with tile.TileContext(nc) as tc:
        with (
            tc.tile_pool(name="const", bufs=1) as constp,
            tc.tile_pool(name="zp", bufs=6) as zp,
            tc.tile_pool(name="mp", bufs=6) as mp,
            tc.tile_pool(name="xown", bufs=7) as xownp,
            tc.tile_pool(name="xstr", bufs=4) as xstrp,
            tc.tile_pool(name="x3p", bufs=3) as x3p,
            tc.tile_pool(name="xkT", bufs=3) as xkTp,
            tc.tile_pool(name="outp", bufs=3) as outp,
            tc.tile_pool(name="ps_seg", bufs=4, space="PSUM") as ps_seg,
            tc.tile_pool(name="ps_tp", bufs=2, space="PSUM") as ps_tp,
            tc.tile_pool(name="ps_o", bufs=2, space="PSUM") as ps_o,
        ):
            # ---- preload constants ----
            idxs_sb = constp.tile([128, IDXCOLS], mybir.dt.int16)
            nc.sync.dma_start(idxs_sb[:], idxs_d[:])
            val1_sb = constp.tile([128, NCH], DT)
            nc.sync.dma_start(val1_sb[:], val1_d[:])
            val2_sb = constp.tile([128, NCH], DT)
            nc.sync.dma_start(val2_sb[:], val2_d[:])
            roff_sb = constp.tile([128, NCH], DT)
            nc.sync.dma_start(roff_sb[:], roff_d[:])
            iota_sb = constp.tile([128, 128], DT)
            nc.sync.dma_start(iota_sb[:], iota_d[:])
            w_sb = constp.tile([CIN, K * COUT], DT)
            nc.sync.dma_start(w_sb[:], w_d[:])
            bias_sb = constp.tile([COUT, 1], f32)
            nc.sync.dma_start(bias_sb[:], bias_d[:])
            ident_sb = constp.tile([128, 128], DT)
            nc.sync.dma_start(ident_sb[:], ident_d[:])
            # one register per distinct gather size, reused by all steps.
            # The ucode trims trailing idx=-1 slots at runtime (per core),
            # so the register only needs the compile-time slot count.
            nregs = {nv: nc.gpsimd.to_reg(nv)
                     for nv in sorted({blk.nidx for blk in blocks})}

            # pre-zero the gather buffers (stale SBUF bits could decode as
            # NaN; a NaN surviving in a never-gathered slot would poison
            # the psum through a val=0 M column)
            for _ in range(6):
                zt = zp.tile([128, MAX_BLK, F], DT, tag="z")
                nc.vector.memset(zt[:], 0)

            # k=0,1 einsum partials, (o, v) tiles side by side: [128, T*B*128]
            accA_sb = constp.tile([128, TILES_PER_CORE * B * 128], DT)
            # hi-phase psum partials, (v, f) tiles side by side
            hpart_sb = constp.tile([128, TILES_PER_CORE * F], DT)

            def spmm_step(step):
                """One A-application; returns nothing (epilogues inline)."""
                half_split = _half_rows()
                x0_tabs = (x0_tab[0:half_split, :], x0_tab[half_split:V_PAD, :])
                if step == 1:
                    tabs = x0_tabs
                    val_sb = val1_sb
                elif step == 2:
                    tabs = x0_tabs if DEBUG_NOCC else (x1_lo[:, :], x1_hi[:, :])
                    val_sb = val2_sb
                else:
                    tabs = x0_tabs if DEBUG_NOCC else (x2_lo[:, :], x2_hi[:, :])
                    val_sb = val2_sb

                psums = {}
                for bi, blk in enumerate(blocks):
                    n = blk.n
                    z = zp.tile([128, MAX_BLK, F], DT, tag="z")
                    nidx = blk.nidx
                    nc.gpsimd.dma_gather(
                        z[:, 0:n, :],
                        tabs[blk.half][:],
                        idxs_sb[:, blk.icol:blk.icol + (nidx + 15) // 16],
                        nidx,
                        nregs[nidx],
                        F,
                        queue_num=blk.queue,
                    )
                    m = mp.tile([128, MAX_BLK, 128], DT, tag="m")
                    nc.vector.tensor_tensor(
                        out=m[:, 0:n, :],
                        in0=roff_sb[:, blk.cg0:blk.cg0 + n, None].to_broadcast(
                            [128, n, 128]),
                        in1=iota_sb[:, None, :].to_broadcast([128, n, 128]),
                        op=mybir.AluOpType.is_equal,
                    )
                    nc.vector.tensor_tensor(
                        out=m[:, 0:n, :],
                        in0=m[:, 0:n, :],
                        in1=val_sb[:, blk.cg0:blk.cg0 + n, None].to_broadcast(
                            [128, n, 128]),
                        op=mybir.AluOpType.mult,
                    )
                    for j, (t, first, last) in enumerate(blk.chunks):
                        if first:
                            psums[t] = ps_seg.tile([128, F], f32, tag="seg",
                                                   name="seg")
                        pc = min(128, nidx - j * 128)
                        nc.tensor.matmul(
                            psums[t][:],
                            lhsT=m[0:pc, j, :],
                            rhs=z[0:pc, j, :],
                            start=first,
                            stop=last,
                        )
                        if last:
                            if blk.half == 1:
                                # hi phase done for this tile: spill the
                                # partial; the lo phase re-opens a psum and
                                # the epilogue merges the two
                                nc.scalar.activation(
                                    hpart_sb[:, t * F:(t + 1) * F],
                                    psums.pop(t)[:],
                                    mybir.ActivationFunctionType.Copy)
                            else:
                                epilogue(step, t, psums.pop(t))

            rb = _quarter_rows()
            ag_tiles = {int(rb[q + 1]) // 128 - 1: q for q in range(len(rb) - 1)}

            def maybe_ag(step, t):
                # emit this quarter's AllGather right after its last tile's
                # epilogue so it runs on the collectives engine while the
                # remaining tiles' gathers proceed
                if DEBUG_NOCC or step == 3 or t not in ag_tiles:
                    return
                q = ag_tiles[t]
                if step == 1:
                    own_d, tlo, thi = x1_own_d, x1_lo, x1_hi
                else:
                    own_d, tlo, thi = x2_own_d, x2_lo, x2_hi
                lo, hi = int(rb[q]), int(rb[q + 1])
                hs = _half_rows()
                o0, o1 = NCORES * lo, NCORES * hi
                dest = tlo[o0:o1, :] if o1 <= hs else thi[o0 - hs:o1 - hs, :]
                nc.gpsimd.collective_compute(
                    "AllGather", mybir.AluOpType.bypass, replica_groups=rg,
                    ins=[own_d[lo:hi, :].opt()], outs=[dest.opt()])

            def epilogue(step, t, psum):
                sl = slice(t * 128, (t + 1) * 128)
                hp = hpart_sb[:, t * F:(t + 1) * F]
                if step == 1:
                    xo = xownp.tile([128, F], DT, tag="xo")
                    nc.vector.tensor_add(out=xo[:], in0=psum[:], in1=hp)
                    nc.sync.dma_start(x1_own_d[sl, :], xo[:])
                    maybe_ag(step, t)
                    einsum_a(t, xo)
                elif step == 2:
                    x0t = xstrp.tile([128, F], DT, tag="xs")
                    nc.sync.dma_start(x0t[:], x0_own[sl, :])
                    xo = xownp.tile([128, F], DT, tag="xo")
                    nc.vector.tensor_sub(out=xo[:], in0=psum[:], in1=x0t[:])
                    nc.vector.tensor_add(out=xo[:], in0=xo[:], in1=hp)
                    nc.sync.dma_start(x2_own_d[sl, :], xo[:])
                    maybe_ag(step, t)
                else:
                    x1t = xstrp.tile([128, F], DT, tag="xs")
                    nc.sync.dma_start(x1t[:], x1_own_d[sl, :])
                    x3t = x3p.tile([128, F], DT, tag="x3")
                    nc.vector.tensor_sub(out=x3t[:], in0=psum[:], in1=x1t[:])
                    nc.vector.tensor_add(out=x3t[:], in0=x3t[:], in1=hp)
                    einsum_b(t, x3t)

            def kterm(po, src, k, start, stop):
                tp = ps_tp.tile([128, 128], DT, tag="tp", name="tp")
                nc.tensor.transpose(tp[:], src, ident_sb[:])
                xkT = xkTp.tile([128, 128], DT, tag="xkT", name="xkT")
                nc.vector.tensor_copy(out=xkT[:], in_=tp[:])
                nc.tensor.matmul(
                    po[:], lhsT=w_sb[:, k * COUT:(k + 1) * COUT], rhs=xkT[:],
                    start=start, stop=stop)

            def einsum_a(t, x1t):
                # k=0,1 terms (+bias), run in step 1's shadow; result parked
                # in SBUF until einsum_b combines it.
                sl = slice(t * 128, (t + 1) * 128)
                x0t = xstrp.tile([128, F], DT, tag="xs")
                nc.sync.dma_start(x0t[:], x0_own[sl, :])
                for b in range(B):
                    po = ps_o.tile([128, 128], mybir.dt.float32, tag="po",
                                   name="po")
                    kterm(po, x0t[:, b * 128:(b + 1) * 128], 0, True, False)
                    kterm(po, x1t[:, b * 128:(b + 1) * 128], 1, False, True)
                    nc.scalar.activation(
                        accA_sb[:, (t * B + b) * 128:(t * B + b + 1) * 128],
                        po[:], mybir.ActivationFunctionType.Identity,
                        bias=bias_sb[:, 0:1])

            def einsum_b(t, x3t):
                sl = slice(t * 128, (t + 1) * 128)
                x2t = xstrp.tile([128, F], DT, tag="xs")
                nc.sync.dma_start(x2t[:], x2_own_d[sl, :])
                for b in range(B):
                    po = ps_o.tile([128, 128], mybir.dt.float32, tag="po",
                                   name="po")
                    kterm(po, x2t[:, b * 128:(b + 1) * 128], 2, True, False)
                    kterm(po, x3t[:, b * 128:(b + 1) * 128], 3, False, True)
                    ob = outp.tile([128, 128], mybir.dt.float32, tag="ob")
                    nc.vector.tensor_add(
                        out=ob[:], in0=po[:],
                        in1=accA_sb[:, (t * B + b) * 128:(t * B + b + 1) * 128])
                    nc.sync.dma_start(out_d[b, :, t * 128:(t + 1) * 128], ob[:])

            spmm_step(1)
            if DEBUG_STEPS >= 2:
                spmm_step(2)
            if DEBUG_STEPS >= 3:
                spmm_step(3)

    nc.compile()
    return nc


# ---------------------------------------------------------------------------
# entry point
# ---------------------------------------------------------------------------

def kernel(x, edge_row, edge_col, edge_vals, weights, biases):
    global LAST_RESULT
    x = np.asarray(x, dtype=np.float32)
    edge_row = np.asarray(edge_row, dtype=np.int32)
    edge_col = np.asarray(edge_col, dtype=np.int32)
    edge_vals = np.asarray(edge_vals, dtype=np.float32)
    weights = np.asarray(weights, dtype=np.float32)
    biases = np.asarray(biases, dtype=np.float32)

    blocks, NCH, IDXCOLS, cnts, per_core = _preprocess(
        edge_row.astype(np.int64), edge_col.astype(np.int64), edge_vals)

    # node-major feature table (V_PAD, F), b-major features, rows permuted
    # to the (quarter, core, offset) AllGather-chunk order
    x0 = np.transpose(x, (2, 0, 1)).reshape(V, F)
    x0n = np.zeros((V_PAD, F), dtype=np.float32)
    x0n[:V] = x0
    x0p = np.empty_like(x0n)
    x0p[_table_row(np.arange(V_PAD))] = x0n
    x0p = x0p.astype(NPDT)
    x0n = x0n.astype(NPDT)

    w_host = np.transpose(weights, (1, 0, 2)).reshape(CIN, K * COUT).astype(NPDT)
    bias_host = biases.reshape(COUT, 1).astype(np.float32)
    iota_host = np.broadcast_to(
        np.arange(128, dtype=np.float32)[None, :], (128, 128)).astype(NPDT).copy()
    ident_host = np.eye(128, dtype=np.float32).astype(NPDT)

    nc = _build_nc(blocks, NCH, IDXCOLS)

    in_maps = []
    for c in range(NCORES):
        idx_np, val_np, roff_np = per_core[c]
        in_maps.append({
            "x0_tab": x0p,
            "x0_own": x0n[c * R:(c + 1) * R].copy(),
            "idxs": idx_np,
            "val1": val_np.astype(NPDT),
            "val2": (2.0 * val_np).astype(NPDT),
            "roff": roff_np.astype(NPDT),
            "iota": iota_host,
            "ident": ident_host,
            "w": w_host,
            "bias": bias_host,
        })

    res = run_bass_kernel_spmd(nc, in_maps, list(range(NCORES)))
    LAST_RESULT = res
    out = np.concatenate([res.results[c]["out"] for c in range(NCORES)], axis=2)
    return np.ascontiguousarray(out[:, :, :V]).astype(np.float32)



# revision 43
# speedup vs baseline: 1.2819x; 1.2819x over previous
"""ChebConv (K=4) distributed Trainium2 kernel — 8 NeuronCores.

Strategy:
  - x (B,Cin,V) is reshaped host-side to a node-major feature table
    x0[(V, F=B*Cin)] (bf16) and replicated to all 8 cores as the gather
    table for spmm step 1.
  - Edges are sorted by destination row; rows are sharded across the 8
    cores (6272 rows each, V padded to 50176). Each core processes only
    the edges landing in its rows.
  - Each spmm step: dma_gather (MoE gather instruction, 1KB descriptors)
    pulls x[col] rows for chunks of 128 edges; a per-chunk segment
    matrix M[e, r] = val[e] * (row[e] == r) is built on the vector
    engine; PE matmul M^T @ z accumulates each 128-row tile in PSUM.
    The Chebyshev recurrence (x2 = 2*A@x1 - x0) folds the factor 2 into
    the edge values and becomes a single subtract in the epilogue.
  - After steps 1 and 2 an 8-core AllGather rebuilds the full node table
    (the "halo exchange") used as the next step's gather source.
  - The K-contraction einsum runs per row-tile right after step 3:
    PE-transpose of each (v,c) tile then matmul against the replicated
    weights, bias added on the scalar engine, f32 result DMA'd out.

int16 gather indices limit a table to 32768 rows, so the node table is
split in two halves at a tile-aligned quarter boundary; edges are
grouped host-side by (row-tile, col-half) and padded to multiples of
128. Per-step AllGathers are chunked over tile-aligned quarter row
ranges and emitted inline at quarter boundaries so they overlap the
remaining gathers; the k=0,1 half of the output einsum runs in step 1's
shadow, k=2,3 and the combine in step 3's epilogue.
"""

import os
import numpy as np
import ml_dtypes

import concourse.bacc as bacc
import concourse.bass as bass
import concourse.mybir as mybir
import concourse.tile as tile
import concourse.tile_sem_assignment as tsa
from concourse.bass_utils import run_bass_kernel_spmd

# Tile assigns DMASW semaphore lanes round-robin regardless of the SWDGE
# queue an instruction runs on. With multi-queue gathers that mixes
# completions from different queues on one lane semaphore (out-of-order
# increments vs. the lane's FIFO wait thresholds). Pin queue-tagged Pool
# DMAs to lanes {2q, 2q+1} so every lane semaphore is fed by exactly one
# queue (per-queue completions are FIFO).
if not getattr(tsa.TileClockTick, "_cheb_queue_lanes", False):
    _orig_assign_tick = tsa.TileClockTick._assign_tick

    def _assign_tick_queue_aware(self, inst):
        q = getattr(inst, "queue_num", None)
        if (
            # multi-queue mode pins every queue (incl. 0); single-queue
            # keeps Tile's native lane rotation for queue 0
            (q or (q == 0 and NQUEUES > 1))
            and inst.engine == mybir.EngineType.Pool
            and isinstance(inst, tsa.DMAInst)
        ):
            toggles = getattr(self, "_cheb_q_toggle", None)
            if toggles is None:
                toggles = self._cheb_q_toggle = {}
            t = toggles.get(q, 0)
            toggles[q] = t ^ 1
            lane = (2 * q + t) % self.swdge_sem_count
            saved = self.next_sw_dma_idx
            self.next_sw_dma_idx = lane
            try:
                return _orig_assign_tick(self, inst)
            finally:
                self.next_sw_dma_idx = saved
        return _orig_assign_tick(self, inst)

    tsa.TileClockTick._assign_tick = _assign_tick_queue_aware
    tsa.TileClockTick._cheb_queue_lanes = True

# ----- problem constants (hardcoded per spec) -----
V = 50000
B = 4
CIN = 128
COUT = 128
K = 4
E = 800000
F = B * CIN  # 512
NCORES = 8
TILES_PER_CORE = 49
R = TILES_PER_CORE * 128          # 6272 rows per core
V_PAD = NCORES * R                # 50176
# lo/hi gather-table split (int16 idx limit) computed from quarter bounds

MAX_BLK = int(os.environ.get("CHEB_MAXBLK", "8"))  # chunks per gather block
NQUEUES = int(os.environ.get("CHEB_NQ", "1"))      # SWDGE queues for gathers
SCRATCH = int(os.environ.get("CHEB_SCRATCH", "16384"))  # SWDGE ring bytes/queue
# Descriptor trimming: pad slots get idx=-1 and the gather's num_idxs
# register is loaded with the per-core VALID count at runtime, so padded
# slots generate no descriptors and no HBM reads.  The register MUST
# match the count of non-negative idxs exactly: the instruction decode
# reserves descriptor-ring space from the register while the ucode
# pushes one descriptor per valid idx — a mismatch leaks ring slots
# until the queue deadlocks (observed on HW with slot-count registers).
TRIM = os.environ.get("CHEB_NOTRIM", "") != "1"


def _quarter_rows():
    """Tile-aligned AllGather chunk boundaries (local rows).

    The last chunk is deliberately small: it is emitted after the step's
    final epilogue and directly gates the next step's hi-half gathers.
    """
    if TILES_PER_CORE == 49:
        # lo half = first two quarters = 32 tiles = exactly 32768 table
        # rows (max int16-addressable); tiny last quarter so the final,
        # boundary-gating AllGather is short
        qb = [0, 16 * 128, 32 * 128, 45 * 128, 49 * 128]
        return np.array(qb, dtype=np.int64)
    nag = min(4, TILES_PER_CORE)
    base, rem = divmod(TILES_PER_CORE, nag)
    qb = [0]
    for q in range(nag):
        qb.append(qb[-1] + (base + (1 if q < rem else 0)) * 128)
    return np.array(qb, dtype=np.int64)


def _half_rows():
    """Gather-table row where the lo/hi (int16) table split falls."""
    rb = _quarter_rows()
    return NCORES * int(rb[(len(rb) - 1 + 1) // 2])


def _table_row(v):
    """Node id -> gather-table row.

    The per-step AllGather runs as chunked collectives over tile-aligned
    quarter row ranges. Each chunk's output is rank-major and must be
    contiguous, so the table row order is (quarter, core, offset).
    """
    rb = _quarter_rows()
    r = v // R
    l = v % R
    q = np.searchsorted(rb, l, side="right") - 1
    qlen = rb[q + 1] - rb[q]
    return NCORES * rb[q] + r * qlen + (l - rb[q])
DEBUG_NOCC = os.environ.get("CHEB_NOCC", "") == "1"    # skip collectives
DEBUG_STEPS = int(os.environ.get("CHEB_STEPS", "3"))   # spmm steps to run

USE_BF16 = os.environ.get("CHEB_F32", "") != "1"
DT = mybir.dt.bfloat16 if USE_BF16 else mybir.dt.float32
NPDT = ml_dtypes.bfloat16 if USE_BF16 else np.float32

LAST_RESULT = None  # test harness reads exec_time_ns from here


# ---------------------------------------------------------------------------
# host-side edge preprocessing
# ---------------------------------------------------------------------------

class _Block:
    __slots__ = ("half", "icol", "cg0", "n", "chunks", "nidx", "queue")

    def __init__(self, half, icol, cg0, n, chunks, nidx):
        self.queue = 0
        self.half = half      # 0 = cols [0, HALF), 1 = cols [HALF, V_PAD)
        self.icol = icol      # column offset into the idx sbuf tensor
        self.cg0 = cg0        # first global chunk id of this block
        self.n = n            # number of 128-edge chunks
        self.chunks = chunks  # list of (tile, is_first_of_tile, is_last_of_tile)
        self.nidx = nidx      # index slots (<= n*128); actual gathered count
                              # comes from the per-core counts register


def _preprocess(edge_row, edge_col, edge_vals):
    """Group/pad edges per (core, row-tile, col-half).

    Chunk counts are equalized across cores (max) so all cores run the
    same instruction graph.  One gather block per (tile, half) bucket:
    short cores pad with trailing idx=-1 slots, which the gather ucode
    trims (no descriptors generated); the per-core valid count is
    shipped in `cnts` and loaded into the gather's num_idxs register at
    runtime.  Returns (blocks, NCH, IDXCOLS, cnts, per-core arrays).
    """
    core = edge_row // R
    tile_id = (edge_row % R) // 128
    tcol = _table_row(edge_col)
    half_split = _half_rows()
    half = (tcol >= half_split).astype(np.int64)

    # bucket edges by (core, tile, half)
    key = (core * TILES_PER_CORE + tile_id) * 2 + half
    order = np.argsort(key, kind="stable")
    skey = key[order]
    srow = edge_row[order]
    scol = tcol[order]
    sval = edge_vals[order]
    nkeys = NCORES * TILES_PER_CORE * 2
    counts = np.bincount(skey, minlength=nkeys).reshape(NCORES, TILES_PER_CORE, 2)
    starts = np.zeros(nkeys + 1, dtype=np.int64)
    np.cumsum(counts.reshape(-1), out=starts[1:])

    # chunks per (tile, half): max over cores, >=1 each
    nchunk = np.ceil(counts / 128.0).astype(np.int64).max(axis=0)  # (T, 2)
    nchunk = np.maximum(nchunk, 1)

    # Phase-major chunk order: ALL hi-col blocks first (their gather
    # table is the one the previous step's late AllGather chunks feed —
    # by the time this step starts they have landed), then all lo-col
    # blocks.  Within each phase, groups owning the hi-table rows
    # (tiles >= 32) come first so their AllGather chunks fire mid-step
    # and land before the NEXT step's hi phase.  Tiles keep 4-tile
    # groups so the psum pool rotation stays within its 4 buffers.
    # Buckets larger than BLK_CAP chunks split into parts (descriptor
    # ring capacity caps one gather at ~1024 descriptors).
    tile_seq = list(range(32, TILES_PER_CORE)) + list(range(0, 32))
    order = [(t, h) for h in (1, 0) for t in tile_seq]
    chunk_list = []    # (tile, half) per global chunk, in order
    part_list = {}     # (t, h) -> list of (cg0, part_chunks, block_idx)
    blocks = []
    if TRIM:
        # one gather block per bucket part so each core's pad slots are
        # trailing within their block (trimmable via idx=-1 + count reg)
        for (t, h) in order:
            n = int(nchunk[t, h])
            parts = []
            for p0 in range(0, n, MAX_BLK):
                pn = min(MAX_BLK, n - p0)
                cg0 = len(chunk_list)
                parts.append((cg0, pn, len(blocks)))
                chunk_list.extend([(t, h)] * pn)
                blocks.append(_Block(h, cg0 * 8, cg0, pn, [t] * pn,
                                     pn * 128))
            part_list[(t, h)] = parts
    else:
        for (t, h) in order:
            n = int(nchunk[t, h])
            cg0 = len(chunk_list)
            part_list[(t, h)] = [(cg0, n, -1)]
            chunk_list.extend([(t, h)] * n)
        # cut each phase's chunk run into MAX_BLK-sized gather blocks
        i = 0
        while i < len(chunk_list):
            h = chunk_list[i][1]
            j = i
            while (j < len(chunk_list) and j - i < MAX_BLK
                   and chunk_list[j][1] == h):
                j += 1
            blocks.append(_Block(h, i * 8, i, j - i,
                                 [chunk_list[k][0] for k in range(i, j)],
                                 (j - i) * 128))
            i = j

    NCH = len(chunk_list)
    IDXCOLS = NCH * 8

    # greedy queue assignment: keep the 4 Q7 pairs' descriptor loads
    # balanced in every 4-block window (round-robin would pair small
    # hi buckets onto the same queue)
    qload = [0] * max(NQUEUES, 1)
    for blk in blocks:
        q = min(range(len(qload)), key=lambda i: qload[i])
        blk.queue = q
        qload[q] += blk.n

    # mark first/last chunk per (tile, half) bucket: `first` opens the
    # tile's psum for the phase, `last` closes it (hi: partial spill,
    # lo: full epilogue)
    bucket_first = {k: parts[0][0] for k, parts in part_list.items()}
    bucket_last = {k: parts[-1][0] + parts[-1][1] - 1
                   for k, parts in part_list.items()}
    for blk in blocks:
        marked = []
        h = blk.half
        for j, t in enumerate(blk.chunks):
            i = blk.cg0 + j
            marked.append((t, i == bucket_first[(t, h)],
                           i == bucket_last[(t, h)]))
        blk.chunks = marked

    # per-core packed arrays
    per_core = []
    cnts = np.zeros((NCORES, len(blocks)), dtype=np.int32)
    for c in range(NCORES):
        idx_np = np.zeros((128, IDXCOLS), dtype=np.int16)
        val_np = np.zeros((128, NCH), dtype=np.float32)
        roff_np = np.zeros((128, NCH), dtype=np.float32)
        for t in range(TILES_PER_CORE):
            base_row = c * R + t * 128
            for h in range(2):
                kidx = (c * TILES_PER_CORE + t) * 2 + h
                s, e = starts[kidx], starts[kidx + 1]
                col = scol[s:e].astype(np.int64) - h * half_split
                row = srow[s:e].astype(np.int64) - base_row
                val = sval[s:e].astype(np.float32)
                pos = 0
                for cg0p, pn, bix in part_list[(t, h)]:
                    cap = pn * 128
                    ccol = col[pos:pos + cap]
                    crow = row[pos:pos + cap]
                    cval = val[pos:pos + cap]
                    pos += len(ccol)
                    cnt = len(ccol)
                    if cnt == 0:
                        # gather one dummy row so num_idxs stays > 0
                        ccol = np.zeros(1, dtype=np.int64)
                        crow = np.zeros(1, dtype=np.int64)
                        cval = np.zeros(1, dtype=np.float32)
                        cnt = 1
                    pad = cap - cnt
                    if pad:
                        # trailing -1 slots generate no descriptors (the
                        # count register is loaded with `cnt` at runtime)
                        ccol = np.concatenate(
                            [ccol, np.full(pad, -1 if TRIM else 0,
                                           dtype=np.int64)])
                        crow = np.concatenate(
                            [crow, np.zeros(pad, dtype=np.int64)])
                        cval = np.concatenate(
                            [cval, np.zeros(pad, dtype=np.float32)])
                    per_core_write(idx_np, val_np, roff_np, cg0p,
                                   ccol, crow, cval)
                    if bix >= 0:
                        cnts[c, bix] = cnt
        per_core.append((idx_np, val_np, roff_np))
    return blocks, NCH, IDXCOLS, cnts, per_core


def per_core_write(idx_np, val_np, roff_np, cg0, col, row, val):
    # idx layout per dma_gather: within a (t,h) group starting at global
    # chunk cg0 (icol = cg0*8), edge i -> col cg0*8 + i//16, partition
    # (i%16) + 16*g replicated across the 8 gpsimd core groups g.
    n128 = col.shape[0]
    i = np.arange(n128)
    cols = cg0 * 8 + i // 16
    parts = i % 16
    for g in range(8):
        idx_np[parts + 16 * g, cols] = col.astype(np.int16)
    # val/rowoff layout: (partition=edge%128, col=global chunk id)
    ch = cg0 + i // 128
    p = i % 128
    val_np[p, ch] = val
    roff_np[p, ch] = row.astype(np.float32)


# ---------------------------------------------------------------------------
# device graph
# ---------------------------------------------------------------------------

def _build_nc(blocks, NCH, IDXCOLS):
    nc = bacc.Bacc("TRN2", target_bir_lowering=False, debug=False,
                   num_devices=NCORES, num_swdge_queues=NQUEUES,
                   dynamic_dma_scratch_size=SCRATCH)
    f32 = mybir.dt.float32
    NBLK = len(blocks)

    # ---- I/O ----
    x0_tab = nc.dram_tensor("x0_tab", [V_PAD, F], DT, kind="ExternalInput")
    x0_own = nc.dram_tensor("x0_own", [R, F], DT, kind="ExternalInput")
    idxs_d = nc.dram_tensor("idxs", [128, IDXCOLS], mybir.dt.int16,
                            kind="ExternalInput")
    cnts_d = nc.dram_tensor("cnts", [1, NBLK], mybir.dt.int32,
                            kind="ExternalInput")
    val1_d = nc.dram_tensor("val1", [128, NCH], DT, kind="ExternalInput")
    val2_d = nc.dram_tensor("val2", [128, NCH], DT, kind="ExternalInput")
    roff_d = nc.dram_tensor("roff", [128, NCH], DT, kind="ExternalInput")
    iota_d = nc.dram_tensor("iota", [128, 128], DT, kind="ExternalInput")
    ident_d = nc.dram_tensor("ident", [128, 128], DT, kind="ExternalInput")
    w_d = nc.dram_tensor("w", [CIN, K * COUT], DT, kind="ExternalInput")
    bias_d = nc.dram_tensor("bias", [COUT, 1], f32, kind="ExternalInput")
    out_d = nc.dram_tensor("out", [B, COUT, R], f32, kind="ExternalOutput")

    # ---- internal DRAM ----
    x1_own_d = nc.dram_tensor("x1_own_d", [R, F], DT)
    x2_own_d = nc.dram_tensor("x2_own_d", [R, F], DT)
    # lo/hi halves as separate tensors so lo-half gathers only depend on
    # the AllGather chunks that write them (Tile tracks DRAM deps per
    # tensor, not per range)
    HS = _half_rows()
    x1_lo = nc.dram_tensor("x1_lo", [HS, F], DT, addr_space="Shared")
    x1_hi = nc.dram_tensor("x1_hi", [V_PAD - HS, F], DT, addr_space="Shared")
    x2_lo = nc.dram_tensor("x2_lo", [HS, F], DT, addr_space="Shared")
    x2_hi = nc.dram_tensor("x2_hi", [V_PAD - HS, F], DT, addr_space="Shared")

    rg = [list(range(NCORES))]

    with tile.TileContext(nc) as tc:
        with (
            tc.tile_pool(name="const", bufs=1) as constp,
            tc.tile_pool(name="zp", bufs=6) as zp,
            tc.tile_pool(name="mp", bufs=6) as mp,
            tc.tile_pool(name="xown", bufs=7) as xownp,
            tc.tile_pool(name="xstr", bufs=4) as xstrp,
            tc.tile_pool(name="x3p", bufs=3) as x3p,
            tc.tile_pool(name="xkT", bufs=3) as xkTp,
            tc.tile_pool(name="outp", bufs=3) as outp,
            tc.tile_pool(name="ps_seg", bufs=4, space="PSUM") as ps_seg,
            tc.tile_pool(name="ps_tp", bufs=2, space="PSUM") as ps_tp,
            tc.tile_pool(name="ps_o", bufs=2, space="PSUM") as ps_o,
        ):
            # ---- preload constants ----
            idxs_sb = constp.tile([128, IDXCOLS], mybir.dt.int16)
            nc.sync.dma_start(idxs_sb[:], idxs_d[:])
            val1_sb = constp.tile([128, NCH], DT)
            nc.sync.dma_start(val1_sb[:], val1_d[:])
            val2_sb = constp.tile([128, NCH], DT)
            nc.sync.dma_start(val2_sb[:], val2_d[:])
            roff_sb = constp.tile([128, NCH], DT)
            nc.sync.dma_start(roff_sb[:], roff_d[:])
            iota_sb = constp.tile([128, 128], DT)
            nc.sync.dma_start(iota_sb[:], iota_d[:])
            w_sb = constp.tile([CIN, K * COUT], DT)
            nc.sync.dma_start(w_sb[:], w_d[:])
            bias_sb = constp.tile([COUT, 1], f32)
            nc.sync.dma_start(bias_sb[:], bias_d[:])
            ident_sb = constp.tile([128, 128], DT)
            nc.sync.dma_start(ident_sb[:], ident_d[:])
            # one register per distinct gather size, reused by all steps
            nregs = {nv: nc.gpsimd.to_reg(nv)
                     for nv in sorted({blk.nidx for blk in blocks})}
            if TRIM:
                cnts_sb = constp.tile([1, NBLK], mybir.dt.int32)
                nc.sync.dma_start(cnts_sb[:], cnts_d[:])
                cnt_regs = [nc.gpsimd.alloc_register(f"cheb_cnt{i}")
                            for i in range(8)]

            # pre-zero the gather buffers (stale SBUF bits could decode as
            # NaN; a NaN surviving in a never-gathered slot would poison
            # the psum through a val=0 M column)
            for _ in range(6):
                zt = zp.tile([128, MAX_BLK, F], DT, tag="z")
                nc.vector.memset(zt[:], 0)

            # k=0,1 einsum partials, (o, v) tiles side by side: [128, T*B*128]
            accA_sb = constp.tile([128, TILES_PER_CORE * B * 128], DT)
            # hi-phase psum partials, (v, f) tiles side by side
            hpart_sb = constp.tile([128, TILES_PER_CORE * F], DT)

            def spmm_step(step):
                """One A-application; returns nothing (epilogues inline)."""
                half_split = _half_rows()
                x0_tabs = (x0_tab[0:half_split, :], x0_tab[half_split:V_PAD, :])
                if step == 1:
                    tabs = x0_tabs
                    val_sb = val1_sb
                elif step == 2:
                    tabs = x0_tabs if DEBUG_NOCC else (x1_lo[:, :], x1_hi[:, :])
                    val_sb = val2_sb
                else:
                    tabs = x0_tabs if DEBUG_NOCC else (x2_lo[:, :], x2_hi[:, :])
                    val_sb = val2_sb

                psums = {}
                for bi, blk in enumerate(blocks):
                    n = blk.n
                    z = zp.tile([128, MAX_BLK, F], DT, tag="z")
                    nidx = blk.nidx
                    if TRIM:
                        if bi % 8 == 0:
                            w = min(8, len(blocks) - bi)
                            nc.gpsimd.reg_load(
                                cnt_regs[:w], cnts_sb[0:1, bi:bi + w])
                        creg = cnt_regs[bi % 8]
                    else:
                        creg = nregs[nidx]
                    nc.gpsimd.dma_gather(
                        z[:, 0:n, :],
                        tabs[blk.half][:],
                        idxs_sb[:, blk.icol:blk.icol + (nidx + 15) // 16],
                        nidx,
                        creg,
                        F,
                        queue_num=blk.queue,
                    )
                    m = mp.tile([128, MAX_BLK, 128], DT, tag="m")
                    nc.vector.tensor_tensor(
                        out=m[:, 0:n, :],
                        in0=roff_sb[:, blk.cg0:blk.cg0 + n, None].to_broadcast(
                            [128, n, 128]),
                        in1=iota_sb[:, None, :].to_broadcast([128, n, 128]),
                        op=mybir.AluOpType.is_equal,
                    )
                    nc.vector.tensor_tensor(
                        out=m[:, 0:n, :],
                        in0=m[:, 0:n, :],
                        in1=val_sb[:, blk.cg0:blk.cg0 + n, None].to_broadcast(
                            [128, n, 128]),
                        op=mybir.AluOpType.mult,
                    )
                    for j, (t, first, last) in enumerate(blk.chunks):
                        if first:
                            psums[t] = ps_seg.tile([128, F], f32, tag="seg",
                                                   name="seg")
                        pc = min(128, nidx - j * 128)
                        nc.tensor.matmul(
                            psums[t][:],
                            lhsT=m[0:pc, j, :],
                            rhs=z[0:pc, j, :],
                            start=first,
                            stop=last,
                        )
                        if last:
                            if blk.half == 1:
                                # hi phase done for this tile: spill the
                                # partial; the lo phase re-opens a psum and
                                # the epilogue merges the two
                                nc.scalar.activation(
                                    hpart_sb[:, t * F:(t + 1) * F],
                                    psums.pop(t)[:],
                                    mybir.ActivationFunctionType.Copy)
                            else:
                                epilogue(step, t, psums.pop(t))

            rb = _quarter_rows()
            ag_tiles = {int(rb[q + 1]) // 128 - 1: q for q in range(len(rb) - 1)}

            def maybe_ag(step, t):
                # emit this quarter's AllGather right after its last tile's
                # epilogue so it runs on the collectives engine while the
                # remaining tiles' gathers proceed
                if DEBUG_NOCC or step == 3 or t not in ag_tiles:
                    return
                q = ag_tiles[t]
                if step == 1:
                    own_d, tlo, thi = x1_own_d, x1_lo, x1_hi
                else:
                    own_d, tlo, thi = x2_own_d, x2_lo, x2_hi
                lo, hi = int(rb[q]), int(rb[q + 1])
                hs = _half_rows()
                o0, o1 = NCORES * lo, NCORES * hi
                dest = tlo[o0:o1, :] if o1 <= hs else thi[o0 - hs:o1 - hs, :]
                nc.gpsimd.collective_compute(
                    "AllGather", mybir.AluOpType.bypass, replica_groups=rg,
                    ins=[own_d[lo:hi, :].opt()], outs=[dest.opt()])

            def epilogue(step, t, psum):
                sl = slice(t * 128, (t + 1) * 128)
                hp = hpart_sb[:, t * F:(t + 1) * F]
                if step == 1:
                    xo = xownp.tile([128, F], DT, tag="xo")
                    nc.vector.tensor_add(out=xo[:], in0=psum[:], in1=hp)
                    nc.sync.dma_start(x1_own_d[sl, :], xo[:])
                    maybe_ag(step, t)
                    einsum_a(t, xo)
                elif step == 2:
                    x0t = xstrp.tile([128, F], DT, tag="xs")
                    nc.sync.dma_start(x0t[:], x0_own[sl, :])
                    xo = xownp.tile([128, F], DT, tag="xo")
                    nc.vector.tensor_sub(out=xo[:], in0=psum[:], in1=x0t[:])
                    nc.vector.tensor_add(out=xo[:], in0=xo[:], in1=hp)
                    nc.sync.dma_start(x2_own_d[sl, :], xo[:])
                    maybe_ag(step, t)
                else:
                    x1t = xstrp.tile([128, F], DT, tag="xs")
                    nc.sync.dma_start(x1t[:], x1_own_d[sl, :])
                    x3t = x3p.tile([128, F], DT, tag="x3")
                    nc.vector.tensor_sub(out=x3t[:], in0=psum[:], in1=x1t[:])
                    nc.vector.tensor_add(out=x3t[:], in0=x3t[:], in1=hp)
                    einsum_b(t, x3t)

            def kterm(po, src, k, start, stop):
                tp = ps_tp.tile([128, 128], DT, tag="tp", name="tp")
                nc.tensor.transpose(tp[:], src, ident_sb[:])
                xkT = xkTp.tile([128, 128], DT, tag="xkT", name="xkT")
                nc.vector.tensor_copy(out=xkT[:], in_=tp[:])
                nc.tensor.matmul(
                    po[:], lhsT=w_sb[:, k * COUT:(k + 1) * COUT], rhs=xkT[:],
                    start=start, stop=stop)

            def einsum_a(t, x1t):
                # k=0,1 terms (+bias), run in step 1's shadow; result parked
                # in SBUF until einsum_b combines it.
                sl = slice(t * 128, (t + 1) * 128)
                x0t = xstrp.tile([128, F], DT, tag="xs")
                nc.sync.dma_start(x0t[:], x0_own[sl, :])
                for b in range(B):
                    po = ps_o.tile([128, 128], mybir.dt.float32, tag="po",
                                   name="po")
                    kterm(po, x0t[:, b * 128:(b + 1) * 128], 0, True, False)
                    kterm(po, x1t[:, b * 128:(b + 1) * 128], 1, False, True)
                    nc.scalar.activation(
                        accA_sb[:, (t * B + b) * 128:(t * B + b + 1) * 128],
                        po[:], mybir.ActivationFunctionType.Identity,
                        bias=bias_sb[:, 0:1])

            def einsum_b(t, x3t):
                sl = slice(t * 128, (t + 1) * 128)
                x2t = xstrp.tile([128, F], DT, tag="xs")
                nc.sync.dma_start(x2t[:], x2_own_d[sl, :])
                for b in range(B):
                    po = ps_o.tile([128, 128], mybir.dt.float32, tag="po",
                                   name="po")
                    kterm(po, x2t[:, b * 128:(b + 1) * 128], 2, True, False)
                    kterm(po, x3t[:, b * 128:(b + 1) * 128], 3, False, True)
                    ob = outp.tile([128, 128], mybir.dt.float32, tag="ob")
                    nc.vector.tensor_add(
                        out=ob[:], in0=po[:],
                        in1=accA_sb[:, (t * B + b) * 128:(t * B + b + 1) * 128])
                    nc.sync.dma_start(out_d[b, :, t * 128:(t + 1) * 128], ob[:])

            spmm_step(1)
            if DEBUG_STEPS >= 2:
                spmm_step(2)
            if DEBUG_STEPS >= 3:
                spmm_step(3)

    nc.compile()
    return nc


# ---------------------------------------------------------------------------
# entry point
# ---------------------------------------------------------------------------

def kernel(x, edge_row, edge_col, edge_vals, weights, biases):
    global LAST_RESULT
    x = np.asarray(x, dtype=np.float32)
    edge_row = np.asarray(edge_row, dtype=np.int32)
    edge_col = np.asarray(edge_col, dtype=np.int32)
    edge_vals = np.asarray(edge_vals, dtype=np.float32)
    weights = np.asarray(weights, dtype=np.float32)
    biases = np.asarray(biases, dtype=np.float32)

    blocks, NCH, IDXCOLS, cnts, per_core = _preprocess(
        edge_row.astype(np.int64), edge_col.astype(np.int64), edge_vals)

    # node-major feature table (V_PAD, F), b-major features, rows permuted
    # to the (quarter, core, offset) AllGather-chunk order
    x0 = np.transpose(x, (2, 0, 1)).reshape(V, F)
    x0n = np.zeros((V_PAD, F), dtype=np.float32)
    x0n[:V] = x0
    x0p = np.empty_like(x0n)
    x0p[_table_row(np.arange(V_PAD))] = x0n
    x0p = x0p.astype(NPDT)
    x0n = x0n.astype(NPDT)

    w_host = np.transpose(weights, (1, 0, 2)).reshape(CIN, K * COUT).astype(NPDT)
    bias_host = biases.reshape(COUT, 1).astype(np.float32)
    iota_host = np.broadcast_to(
        np.arange(128, dtype=np.float32)[None, :], (128, 128)).astype(NPDT).copy()
    ident_host = np.eye(128, dtype=np.float32).astype(NPDT)

    nc = _build_nc(blocks, NCH, IDXCOLS)

    in_maps = []
    for c in range(NCORES):
        idx_np, val_np, roff_np = per_core[c]
        in_maps.append({
            "x0_tab": x0p,
            "x0_own": x0n[c * R:(c + 1) * R].copy(),
            "idxs": idx_np,
            "cnts": cnts[c:c + 1, :].copy(),
            "val1": val_np.astype(NPDT),
            "val2": (2.0 * val_np).astype(NPDT),
            "roff": roff_np.astype(NPDT),
            "iota": iota_host,
            "ident": ident_host,
            "w": w_host,
            "bias": bias_host,
        })

    res = run_bass_kernel_spmd(nc, in_maps, list(range(NCORES)))
    LAST_RESULT = res
    out = np.concatenate([res.results[c]["out"] for c in range(NCORES)], axis=2)
    return np.ascontiguousarray(out[:, :, :V]).astype(np.float32)

